# revision 3
# baseline (speedup 1.0000x reference)
"""BitPackedLinear Trainium2 kernel (8-core SPMD, token-sharded, fp8 DoubleRow).

y = x @ W.T + bias, W = unpack_bits(packed_weight) in {-1,+1}, shapes:
  x [2, 2048, 4096] f32, packed_weight [4096, 512] u8, bias [4096] f32.

Sharding: data-parallel over tokens (4096 tokens -> 512/core). Each core
computes y_c = x_c @ W.T + bias for its token shard against the full
weight; the host just concatenates shards.

Device algorithm per core:
  - W = 2B - 1, so y = 2*(x@B.T) - rowsum(x) + bias. The matmul runs on
    B2 = 2B in {0,2} (exact in fp8 e4m3, pattern 0x40).
  - x is split hi/lo: xh = e4m3(x_bf16), xl = e4m3(x_bf16 - xh). The pair
    (xh + xl) carries ~14 mantissa bits, so fp8 DoubleRow matmuls
    (both operands e4m3, 2 k-rows per PE cell, 0.5 cyc/row) reproduce the
    bf16 result at half the PE time.
  - Pairing avoids duplicating weights: MM j contracts (xh_j, xl_{j+1})
    against (w_j, w_{j+1}); summed over j=0..31 (with wraparound copies
    w_32=w_0, xl-slot 65=xl_0) this equals sum_j w_j*(xh_j + xl_j).
    xT8 is interleaved [hi_0, lo_0, hi_1, lo_1, ...]; the lhsT pair is a
    stride-3 stepped slice; the rhs pair is two adjacent wt slots.
  - Contraction (i) is tiled bit-sliced: i-tile j=(kt,b) = {8*(128*kt+k')+b},
    so every weight tile is one shift+mask from transposed packed bytes.
  - byteT_u8[k',kt,o] = pw[o,128*kt+k'] via PE pass-through transposes
    (batched 4-to-a-PSUM-bank); adjacent o bytes viewed as u16 lanes let
    one tensor_scalar (shl 6-b & 0x4040, or shr 1 for b=7) unpack TWO fp8
    weights per lane at the DVE 4x rate.
  - bias enters each PSUM group via a rank-1 f32r matmul; the rowsum
    correction is applied by the ACT epilogue (Identity + per-partition
    bias = -s_col) which also moves PSUM->SBUF, keeping DVE free.
  - s_col[t] = sum_i x_bf[t,i] via DVE reduces of the bf16 x chunks.
"""
import sys

sys.path.insert(0, "/opt/trn_rl_repo")
from contextlib import ExitStack

import numpy as np

import concourse.tile as tile
from concourse import bacc, mybir
from concourse.bass import ts
from concourse.bass_utils import run_bass_kernel_spmd
from concourse.masks import make_identity

F32 = mybir.dt.float32
F32R = mybir.dt.float32r
BF16 = mybir.dt.bfloat16
U8 = mybir.dt.uint8
U16 = mybir.dt.uint16
FP8 = mybir.dt.float8e4
P = 128

N_CORES = 8
B_DIM, S_DIM, I_DIM, O_DIM = 2, 2048, 4096, 4096
T_FULL = B_DIM * S_DIM          # 4096 tokens
T_SHARD = T_FULL // N_CORES     # 512 tokens per core
OUT_NAME = "y"
OUT_SHAPE = (T_SHARD, O_DIM)

DR = mybir.MatmulPerfMode.DoubleRow
SHL = mybir.AluOpType.logical_shift_left
SHR = mybir.AluOpType.logical_shift_right
AND = mybir.AluOpType.bitwise_and
ADD = mybir.AluOpType.add
SUB = mybir.AluOpType.subtract
MUL = mybir.AluOpType.mult
IDENT = mybir.ActivationFunctionType.Identity


def build(T=T_SHARD, I=I_DIM, O=O_DIM, O_SLAB=512, n_cores=N_CORES):
    assert I % 1024 == 0 and T % P == 0 and O % P == 0 and O % O_SLAB == 0
    KT = I // 1024          # 128-byte groups along i
    NJ = KT * 8             # bit-sliced i-tiles (j = kt*8 + b)
    TT = T // P             # token tiles
    K = I // 8              # packed bytes per weight row
    NSLAB = O // O_SLAB
    OSL_T = O_SLAB // P
    OSL2 = O_SLAB // 2      # u16 pair lanes per slab

    nc = bacc.Bacc("TRN2", target_bir_lowering=False, debug=False,
                   num_devices=n_cores)
    x_d = nc.dram_tensor("x", [T, I], F32, kind="ExternalInput").ap()
    pw_d = nc.dram_tensor("pw", [O, K], U8, kind="ExternalInput").ap()
    bias_d = nc.dram_tensor("bias", [O], F32, kind="ExternalInput").ap()
    y_d = nc.dram_tensor(OUT_NAME, [T, O], F32, kind="ExternalOutput").ap()

    with tile.TileContext(nc) as tc:
        with ExitStack() as ctx:
            const = ctx.enter_context(tc.tile_pool(name="const", bufs=1))
            persist = ctx.enter_context(tc.tile_pool(name="persist", bufs=1))

            ident_bf = const.tile([P, P], BF16)
            make_identity(nc, ident_bf[:])
            ones_r = const.tile([1, P], F32R)
            bias_r = const.tile([1, O], F32R)
            stage = ctx.enter_context(tc.tile_pool(name="stage", bufs=1))

            def emit_bias_stage():
                ones_f32 = stage.tile([1, P], F32)
                nc.vector.memset(ones_f32[:], 1.0)
                nc.vector.tensor_copy(out=ones_r[:], in_=ones_f32[:])
                bias_f32 = stage.tile([1, O], F32)
                nc.sync.dma_start(
                    bias_f32[:], bias_d.rearrange("(b o) -> b o", b=1)
                )
                nc.vector.tensor_copy(out=bias_r[:], in_=bias_f32[:])

            # transposed packed bytes, one u8 per (byte-row k', kt, o);
            # adjacent-o pairs are read back as u16 lanes by the unpack
            byteT = persist.tile([P, KT, O], U8)
            xq_bf = persist.tile([P, NJ, T], BF16)    # transposed bf16 x
            xT8 = persist.tile([P, 2 * NJ + 2, T], FP8)  # hi/lo interleaved
            pw_ap = pw_d.rearrange("(ot p) k -> p ot k", p=P)

            pk_pool = ctx.enter_context(tc.tile_pool(name="pk", bufs=2))
            pkbf_pool = ctx.enter_context(tc.tile_pool(name="pkbf", bufs=2))
            ps_trb = ctx.enter_context(
                tc.tile_pool(name="ps_trb", bufs=2, space="PSUM")
            )
            ps_trx = ctx.enter_context(
                tc.tile_pool(name="ps_trx", bufs=2, space="PSUM")
            )
            xnat_pool = ctx.enter_context(
                tc.tile_pool(name="xnat", bufs=max(2 * TT, KT * TT - 4))
            )
            x32_pool = ctx.enter_context(tc.tile_pool(name="x32", bufs=2))
            scol_pool = ctx.enter_context(tc.tile_pool(name="scol", bufs=2))
            wt_pool = ctx.enter_context(tc.tile_pool(name="wt", bufs=2))
            y_pool = ctx.enter_context(tc.tile_pool(name="ysb", bufs=3))
            ps_mm = ctx.enter_context(
                tc.tile_pool(name="ps_mm", bufs=4, space="PSUM")
            )

            def byte_slab(sl):
                """Fill byteT[:, :, sl*O_SLAB:(sl+1)*O_SLAB] from pw (JIT)."""
                pk = pk_pool.tile([P, OSL_T, K], U8)
                nc.sync.dma_start(pk[:], pw_ap[:, ts(sl, OSL_T), :])
                for otl in range(OSL_T):
                    ot = sl * OSL_T + otl
                    pkbf = pkbf_pool.tile([P, K], BF16)
                    nc.any.tensor_copy(out=pkbf[:], in_=pk[:, otl, :])
                    ps = ps_trb.tile([P, KT, P], BF16, tag="trb_ps")
                    for kt in range(KT):
                        nc.tensor.transpose(
                            ps[:, kt, :], pkbf[:, ts(kt, P)], ident_bf[:]
                        )
                    # one strided copy back: [k', kt, o-block of 128]
                    nc.scalar.copy(out=byteT[:, :, ts(ot, P)], in_=ps[:])

            # slab 0 bytes first: DVE/ACT work exists while x DMAs land
            byte_slab(0)

            # x chunks, kt-major. kt=0 goes via fast HWDGE as f32 + a DVE
            # cast; kt>=1 use SWDGE cast-DMA f32->bf16 (descriptor-gen on
            # the Q7 makes the first cast-DMA land late, so not for kt=0).
            xns = {}
            for kt in range(KT):
                for tt in range(TT):
                    src_ap = x_d[ts(tt, P), ts(kt, 1024)].rearrange(
                        "p (k b) -> p k b", b=8
                    )
                    xn = xnat_pool.tile([P, P, 8], BF16, tag="xn16")
                    if kt == 0:
                        x32 = x32_pool.tile([P, P, 8], F32)
                        nc.sync.dma_start(x32[:], src_ap)
                        nc.vector.tensor_copy(out=xn[:], in_=x32[:])
                    else:
                        nc.gpsimd.dma_start(xn[:], src_ap)
                    xns[kt, tt] = xn

            def unpack_slab(sl):
                """wt slots 0..NJ-1 = weight i-tiles, slot NJ = copy of 0."""
                wt = wt_pool.tile([P, NJ + 1, OSL2], U16)
                for slot in range(NJ + 1):
                    j = slot % NJ
                    kt, b = divmod(j, 8)
                    src = byteT[:, kt, ts(sl, O_SLAB)].bitcast(U16)
                    sh, op = (6 - b, SHL) if b < 7 else (1, SHR)
                    nc.vector.tensor_scalar(
                        out=wt[:, slot, :], in0=src, scalar1=sh,
                        scalar2=0x4040, op0=op, op1=AND,
                    )
                return wt

            wt0 = unpack_slab(0)

            # xT via PE transposes (4 token-tiles batched per PSUM bank),
            # i-tile-major so the matmul stream chases them; hi/lo fp8
            # split immediately after each i-tile lands.
            for kt in range(KT):
                for b in range(8):
                    j = kt * 8 + b
                    ps = ps_trx.tile([P, TT, P], BF16, tag="trx_ps")
                    for tt in range(TT):
                        nc.tensor.transpose(
                            ps[:, tt, :], xns[kt, tt][:, :, b], ident_bf[:]
                        )
                    nc.vector.tensor_copy(out=xq_bf[:, j, :], in_=ps[:])
                    nc.vector.tensor_copy(
                        out=xT8[:, 2 * j, :], in_=xq_bf[:, j, :]
                    )
                    nc.vector.tensor_tensor(
                        out=xT8[:, 2 * j + 1, :], in0=xq_bf[:, j, :],
                        in1=xT8[:, 2 * j, :], op=SUB,
                    )
            # wraparound lo_0 copy (slot 2*NJ+1; slot 2*NJ is a never-read pad)
            nc.vector.tensor_copy(out=xT8[:, 2 * NJ + 1, :], in_=xT8[:, 1, :])

            emit_bias_stage()

            def pair_ap(j, tsub):
                # (hi_j, lo_{j+1}): slots 2j, 2j+3 -> stride-3 stepped slice
                return xT8[:, 2 * j:2 * j + 4:3, ts(tsub, P)]

            # neg_s_col[t] = -sum_i x_bf[t, i] on DVE (consistent with the
            # matmul input); emitted after slab 0's unpack so it doesn't
            # delay the first matmuls
            neg_s = scol_pool.tile([P, TT], F32)
            parts = scol_pool.tile([P, TT, KT], F32, tag="sparts")

            def emit_s_col():
                for tt in range(TT):
                    for kt in range(KT):
                        nc.vector.tensor_reduce(
                            out=parts[:, tt, kt:kt + 1],
                            in_=xns[kt, tt][:],
                            op=ADD,
                            axis=mybir.AxisListType.XY,
                        )
                nc.vector.tensor_reduce(
                    out=neg_s[:],
                    in_=parts[:],
                    op=ADD,
                    axis=mybir.AxisListType.X,
                    negate=True,
                )

            # main o-slab loop
            for sl in range(NSLAB):
                if sl == 0:
                    wt = wt0
                else:
                    byte_slab(sl)
                    wt = unpack_slab(sl)
                wt8 = wt[:].bitcast(FP8)  # [P, NJ+1, O_SLAB]
                if sl == 0:
                    emit_s_col()
                for tsub in range(TT):
                    ps = ps_mm.tile([P, O_SLAB], F32)
                    nc.tensor.matmul(
                        ps[:], ones_r[:], bias_r[:, ts(sl, O_SLAB)],
                        start=True, stop=False,
                    )
                    for j in range(NJ):
                        nc.tensor.matmul(
                            ps[:],
                            pair_ap(j, tsub),
                            wt8[:, j:j + 2, :],
                            start=False, stop=(j == NJ - 1),
                            perf_mode=DR,
                        )
                    y_sb = y_pool.tile([P, O_SLAB], F32)
                    nc.scalar.activation(
                        out=y_sb[:], in_=ps[:], func=IDENT,
                        bias=neg_s[:, tsub:tsub + 1], scale=1.0,
                    )
                    nc.sync.dma_start(
                        y_d[ts(tsub, P), ts(sl, O_SLAB)], y_sb[:]
                    )

    nc.compile()
    return nc


_NC = None


def _get_nc():
    global _NC
    if _NC is None:
        _NC = build()
    return _NC


def run(x, packed_weight, bias, trace=False):
    x = np.ascontiguousarray(np.asarray(x, dtype=np.float32))
    pw = np.ascontiguousarray(np.asarray(packed_weight, dtype=np.uint8))
    bias = np.ascontiguousarray(np.asarray(bias, dtype=np.float32))
    assert x.shape == (B_DIM, S_DIM, I_DIM)
    assert pw.shape == (O_DIM, I_DIM // 8)
    assert bias.shape == (O_DIM,)

    nc = _get_nc()
    xs = x.reshape(T_FULL, I_DIM)
    in_maps = [
        {
            "x": np.ascontiguousarray(xs[c * T_SHARD:(c + 1) * T_SHARD]),
            "pw": pw,
            "bias": bias,
        }
        for c in range(N_CORES)
    ]
    res = run_bass_kernel_spmd(nc, in_maps, list(range(N_CORES)), trace=trace)
    y = np.concatenate(
        [res.results[c][OUT_NAME] for c in range(N_CORES)], axis=0
    )
    return y.reshape(B_DIM, S_DIM, O_DIM), res


def kernel(x, packed_weight, bias):
    y, _ = run(x, packed_weight, bias, trace=False)
    return y
